# revision 39
# baseline (speedup 1.0000x reference)
"""Trainium2 Bass kernel for nn_MemoryBlock (scatter_memory).

Mathematical identity: softmax over the memory-unit axis U produces rows
that sum to exactly 1, so

    out[b] = relu( mean_u( sum_n attn[b,n,u] * V[b,n,:] ) @ Wo + bo )
           = relu( (sum_n X[b,n,:]) @ W2 + c2 )

with W2 = (Wv/U) @ Wo and c2 = (N/U)*bv @ Wo + bo folded on the host --
the whole K/scores/softmax path cancels algebraically, leaving a
memory-bound column-sum of X plus one tiny matmul.

Device-side choices (per core, data-parallel over batch B):
- X is downcast to fp16 on the host (loose tolerance; colsum error is
  ~2e-4), halving HBM traffic to 8.4 MB/core.  The fp16 ones-matmul
  streams 2 cols/cycle, so TensorE consumes the raw chunks directly
  (no pre-reduction needed to keep up with the DMA stream).
- DMAs are strictly 128-partition: measured, non-128 partition counts
  fall off the DGE's optimized engine swizzle and halve the per-engine
  SDMA rate.  Layout is the classic [128 partitions x 64 rows]/batch.
- 16-row chunks = 8 KB per-partition descriptors (4 KB descriptors
  measured ~40% slower per engine).
- Finale per batch: ACT copies the colsum row from PSUM, PE transposes
  it into columns folding the even/odd interleave, one W2 matmul, relu
  with the folded bias, and ACT issues that batch's 512B output DMA
  immediately -- batch 0's output completes mid-stream, only batch 1's
  chain is in the tail.
"""

import contextlib

import numpy as np

B, N, FEAT, MEM, U = 16, 8192, 256, 128, 512
NCORES = 8
BPC = B // NCORES

RPP = N // 128     # 64 rows per partition per batch
# chunk row counts: big 8KB-descriptor chunks, then a small tail so the
# TensorE work left after the last byte lands is a single matmul
CHROWS = [16, 16, 16, 14, 2]
NCH = len(CHROWS)
CWS = [r * FEAT for r in CHROWS]          # fp16 cols per chunk
COFF = [sum(CWS[:j]) for j in range(NCH)]  # col offsets within a batch

_built = None


def _ensure_axon_hooks():
    try:
        import antenv.axon_hooks  # noqa: F401
        return
    except ImportError:
        pass
    import sys
    import types

    m = types.ModuleType("antenv.axon_hooks")
    holder = [None]
    m.set_axon_ntff_profile_hook = lambda h: holder.__setitem__(0, h)
    m.get_axon_ntff_profile_hook = lambda: holder[0]
    sys.modules["antenv.axon_hooks"] = m
    try:
        import antenv

        antenv.axon_hooks = m
    except ImportError:
        pass


def _build():
    import concourse.bacc as bacc
    import concourse.mybir as mybir

    f32 = mybir.dt.float32
    f16 = mybir.dt.float16
    AF = mybir.ActivationFunctionType
    nc = bacc.Bacc(None, enable_partition_id=False, monotonic_sem_count=0)

    X_d = nc.dram_tensor("Xs", [BPC, N, FEAT], f16, kind="ExternalInput")
    # all consts in one line-rate DMA: cols 0:128 = W2 half0, 128:256 =
    # W2 half1, col 256 = c2, padded to 384 cols (1.5KB/partition)
    cst_d = nc.dram_tensor("consts", [128, 384], f32, kind="ExternalInput")
    # outputs padded to 512B/partition: sub-512B HBM writes RMW and the
    # write receipt (which gates the DMA's semaphore) takes 3-7us
    outs_d = [
        nc.dram_tensor(f"out{b}", [MEM, 128], f32, kind="ExternalOutput")
        for b in range(BPC)
    ]

    ctx = contextlib.ExitStack()
    with ctx:
        xts = [
            ctx.enter_context(
                nc.sbuf_tensor(f"xt{b}_{j}", [128, CWS[j]], f16)
            )
            for b in range(BPC)
            for j in range(NCH)
        ]
        ones16 = ctx.enter_context(nc.sbuf_tensor("ones16", [128, 1], f16))
        one_f = ctx.enter_context(nc.sbuf_tensor("one_f", [1, 1], f32))
        cst_sb = ctx.enter_context(nc.sbuf_tensor("cst_sb", [128, 384], f32))
        srows = [
            ctx.enter_context(nc.sbuf_tensor(f"srow{b}", [1, 2 * FEAT], f32))
            for b in range(BPC)
        ]
        stq = ctx.enter_context(nc.sbuf_tensor("stq", [128, 2 * BPC], f32))
        # per-batch padded result rows; col 128*b holds the real output
        res = ctx.enter_context(nc.sbuf_tensor("res", [128, BPC * 128], f32))

        pss = [
            ctx.enter_context(nc.psum_tensor(f"ps{b}", [1, 2 * FEAT], f32))
            for b in range(BPC)
        ]
        pts = ctx.enter_context(nc.psum_tensor("pts", [128, 2 * BPC], f32))
        pso = ctx.enter_context(nc.psum_tensor("pso", [128, BPC], f32))

        dsems = [
            ctx.enter_context(nc.semaphore(f"dsem{i}"))
            for i in range(BPC * NCH)
        ]
        csem = ctx.enter_context(nc.semaphore("csem"))    # const DMAs
        msem = ctx.enter_context(nc.semaphore("msem"))    # ones/one_f memsets
        pesem = ctx.enter_context(nc.semaphore("pesem"))  # PE milestones
        asem = ctx.enter_context(nc.semaphore("asem"))    # ACT srow copies
        vsem = ctx.enter_context(nc.semaphore("vsem"))    # DVE stq copies
        osem = ctx.enter_context(nc.semaphore("osem"))    # output DMAs
        sem_nums = sorted(
            s.num for s in (*dsems, csem, msem, pesem, asem, vsem, osem)
        )

        def xt(b, j):
            return xts[b * NCH + j]

        with nc.Block() as block:

            @block.sync
            def _(sync):
                # even chunks on the SP HWDGE ring (odd chunks ride the ACT
                # ring, so every SDMA engine has a second packet queue to
                # drain across one ring's completion boundaries)
                for b in range(BPC):
                    Xb = X_d[b].rearrange("(p r) f -> p (r f)", p=128)
                    for j in range(0, NCH, 2):
                        sync.dma_start(
                            out=xt(b, j)[:, :],
                            in_=Xb[:, COFF[j] : COFF[j] + CWS[j]],
                        ).then_inc(dsems[b * NCH + j], 16)

            @block.scalar
            def _(scalar):
                # one line-rate const DMA on the ACT ring
                scalar.dma_start(out=cst_sb[:, :], in_=cst_d[:, :]).then_inc(csem, 16)
                # odd chunks on the ACT ring
                for b in range(BPC):
                    Xb = X_d[b].rearrange("(p r) f -> p (r f)", p=128)
                    for j in range(1, NCH, 2):
                        scalar.dma_start(
                            out=xt(b, j)[:, :],
                            in_=Xb[:, COFF[j] : COFF[j] + CWS[j]],
                        ).then_inc(dsems[b * NCH + j], 16)
                scalar.wait_ge(csem, 16)
                # pesem milestones: b0-colsum(1), b1-colsum(2), b0-T(3),
                # b0-W2(4), b1-T(5), b1-W2(6)
                for b in range(BPC):
                    scalar.wait_ge(pesem, b + 1)
                    nc.scalar.activation(
                        out=srows[b][:, :],
                        in_=pss[b][0:1, :],
                        func=AF.Copy,
                        scale=1.0,
                    ).then_inc(asem, 1)
                for b in range(BPC):
                    scalar.wait_ge(pesem, 2 * b + 4)
                    nc.scalar.activation(
                        out=res[:, 128 * b : 128 * b + 1],
                        in_=pso[:, b : b + 1],
                        func=AF.Relu,
                        bias=cst_sb[:, 256:257],
                        scale=1.0,
                    )
                    scalar.dma_start(
                        out=outs_d[b][:, :], in_=res[:, 128 * b : 128 * (b + 1)]
                    ).then_inc(osem, 16)
                scalar.wait_ge(osem, 16 * BPC)

            @block.tensor
            def _(pe):
                pe.wait_ge(msem, 1)
                # both column-sums back to back (the b0 finale's engine
                # round-trips must not delay b1's data consumption)
                for b in range(BPC):
                    k = 0
                    nmm = sum(CWS) // 512
                    lastc = None
                    for j in range(NCH):
                        pe.wait_ge(dsems[b * NCH + j], 16)
                        for m in range(CWS[j] // 512):
                            k += 1
                            lastc = nc.tensor.matmul(
                                pss[b][:, :],
                                lhsT=ones16[:, 0:1],
                                rhs=xt(b, j)[:, (m * 512) : (m + 1) * 512],
                                start=(k == 1),
                                stop=(k == nmm),
                            )
                    lastc.then_inc(pesem, 1)  # b+1
                for b in range(BPC):
                    # fold even/odd halves + transpose into pts columns
                    pe.wait_ge(asem, b + 1)
                    last = None
                    for h in range(2):
                        nc.tensor.matmul(
                            pts[:, 2 * b + h : 2 * b + h + 1],
                            lhsT=srows[b][0:1, h * 128 : (h + 1) * 128],
                            rhs=one_f[0:1, 0:1],
                            is_transpose=True,
                            start=True,
                            stop=False,
                        )
                        last = nc.tensor.matmul(
                            pts[:, 2 * b + h : 2 * b + h + 1],
                            lhsT=srows[b][0:1, 256 + h * 128 : 256 + (h + 1) * 128],
                            rhs=one_f[0:1, 0:1],
                            is_transpose=True,
                            start=False,
                            stop=True,
                        )
                    last.then_inc(pesem, 1)  # 2b+3
                    # out_col[b] = W2^T @ s_feat[b]
                    pe.wait_ge(vsem, b + 1)
                    if b == 0:
                        pe.wait_ge(csem, 16)
                    nc.tensor.matmul(
                        pso[:, b : b + 1],
                        lhsT=cst_sb[:, 0:MEM],
                        rhs=stq[:, 2 * b : 2 * b + 1],
                        start=True,
                        stop=False,
                    )
                    nc.tensor.matmul(
                        pso[:, b : b + 1],
                        lhsT=cst_sb[:, MEM : 2 * MEM],
                        rhs=stq[:, 2 * b + 1 : 2 * b + 2],
                        start=False,
                        stop=True,
                    ).then_inc(pesem, 1)  # 2b+4

            @block.vector
            def _(vector):
                nc.vector.memset(ones16[:, :], 1.0)
                nc.vector.memset(one_f[:, :], 1.0)
                # zero the output padding (the padded out-DMA reads it; the
                # relu write is ordered after this via msem->...->pesem)
                nc.vector.memset(res[:, :], 0.0).then_inc(msem, 1)
                # psum transpose columns -> SBUF for the final matmul rhs
                for b in range(BPC):
                    vector.wait_ge(pesem, 2 * b + 3)
                    nc.vector.tensor_copy(
                        out=stq[:, 2 * b : 2 * b + 2], in_=pts[:, 2 * b : 2 * b + 2]
                    ).then_inc(vsem, 1)

            @block.gpsimd
            def _(gpsimd):
                # once all PE milestones fired, every waiter of the dsems/
                # csem/msem/asem/vsem has already passed and their final
                # increments have landed -- clear them here, hidden under
                # the output DMA's HBM write-receipt window.  pesem/osem
                # may still be waited on by ACT, so they wait for the
                # barrier below.
                gpsimd.wait_ge(pesem, 2 * BPC + 2)
                nc.gpsimd.sem_clear(range(dsems[0].num, msem.num + 1))
                nc.gpsimd.sem_clear(range(asem.num, vsem.num + 1))
                gpsimd.wait_ge(osem, 16 * BPC)

            nc.all_engine_barrier()
            nc.gpsimd.sem_clear(range(pesem.num, pesem.num + 1))
            nc.gpsimd.sem_clear(range(osem.num, osem.num + 1))

    if not nc.is_finalized():
        nc.finalize()
    return nc


def kernel(X, mem, Wk, bk, Wv, bv, Wo, bo):
    global _built
    _ensure_axon_hooks()
    from concourse.bass_utils import run_bass_kernel_spmd

    if _built is None:
        _built = _build()
    nc = _built

    X16 = np.asarray(X).astype(np.float16)
    W2 = (
        (np.asarray(Wv, dtype=np.float64) / float(U))
        @ np.asarray(Wo, dtype=np.float64)
    ).astype(np.float32)
    c2 = (
        np.asarray(bv, dtype=np.float64) * (N / float(U))
    ) @ np.asarray(Wo, dtype=np.float64) + np.asarray(bo, dtype=np.float64)
    consts = np.zeros((128, 384), dtype=np.float32)
    consts[:, 0:128] = W2[0:128]
    consts[:, 128:256] = W2[128:256]
    consts[:, 256] = c2.astype(np.float32)

    in_maps = [
        {
            "Xs": np.ascontiguousarray(X16[i * BPC : (i + 1) * BPC]),
            "consts": consts,
        }
        for i in range(NCORES)
    ]
    r = run_bass_kernel_spmd(nc, in_maps, list(range(NCORES)))
    kernel._last_results = r

    out = np.empty((B, MEM), dtype=np.float32)
    for i in range(NCORES):
        for b in range(BPC):
            out[i * BPC + b] = r.results[i][f"out{b}"][:, 0]
    return out
